# revision 2
# baseline (speedup 1.0000x reference)
# GAT 2-layer message-passing kernel for 8 TRN2 NeuronCores.
# Strategy: destination-node partitioning (12500 dst nodes/core), edges
# sharded by dst owner and grouped into 128-edge tiles per 128-node dst
# block. Scatter/segment-softmax via one-hot matmuls accumulating in PSUM.
# Node features gathered per edge via indirect DMA from an AllGathered
# bf16 table. All floating-point math runs on device.
import sys

sys.path.insert(0, "/opt/trn_rl_repo")

import numpy as np
import ml_dtypes

import concourse.bass as bass
import concourse.bacc as bacc
import concourse.mybir as mybir
import concourse.tile as tile
from concourse.bass import AP, IndirectOffsetOnAxis
from concourse.bass_utils import run_bass_kernel_spmd

F32 = mybir.dt.float32
BF16 = mybir.dt.bfloat16
I32 = mybir.dt.int32
BF = ml_dtypes.bfloat16

N, E, NC = 100000, 1600000, 8
F, H, C1, C2 = 128, 4, 32, 64
NLOC = N // NC            # 12500 dst nodes per core
NPAD = 12544              # 98 * 128
NBLK = NPAD // 128        # 98
NT = NC * NPAD            # global table rows
CHUNK_BLKS = 2            # dst blocks per gather chunk
STAGE = 7                 # build stages (debug bisect)
GATH = 3                  # bitmask: 1=sr gathers, 2=dl gathers
SIM_NOCOLL = False        # replace collectives with local DMAs (TimelineSim)


QR = None  # set below: NT // 4


def _wrap16(flat_i16, ntiles):
    """dma_gather idx layout: logical idx j -> partition j%16, col j//16,
    replicated to all 8 16-partition groups. flat [ntiles*128] -> [128, ntiles*8]."""
    S = ntiles * 8
    a = np.zeros((16, S), np.int16)
    j = np.arange(ntiles * 128)
    a[j % 16, j // 16] = flat_i16
    return np.tile(a, (8, 1))


def _build_shards(edge_index, edge_attr):
    """Shard edges by dst owner, group by (chunk, src-quartile, dst-block),
    pad each (q, b) group to shared tile counts."""
    global QR
    QR = NT // 4
    src = edge_index[0].astype(np.int64)
    dst = edge_index[1].astype(np.int64)
    ea = edge_attr.astype(np.float32)
    owner = dst // NLOC
    dst_loc = dst % NLOC
    src_row = (src // NLOC) * NPAD + (src % NLOC)
    q = src_row // QR
    blk = dst_loc // 128

    counts = np.zeros((NC, 4, NBLK), np.int64)
    np.add.at(counts, (owner, q, blk), 1)
    Tqb = -(-counts.max(0) // 128)  # [4, NBLK] tiles per (q, b)

    # slot ordering: for chunk: for q: for b in chunk: Tqb[q][b] tiles
    nch = -(-NBLK // CHUNK_BLKS)
    tile_off = np.zeros((4, NBLK), np.int64)  # tile offset of each (q, b)
    call_ranges = []  # per chunk: list of 4 (tile_start, tile_end) per q
    blk_segs = [[] for _ in range(NBLK)]  # per block: [(tstart, tend), ...]
    pos = 0
    chunk_list = []
    for c in range(nch):
        b0, b1 = c * CHUNK_BLKS, min((c + 1) * CHUNK_BLKS, NBLK)
        qr = []
        for qq in range(4):
            qs = pos
            for b in range(b0, b1):
                tile_off[qq, b] = pos
                blk_segs[b].append((pos, pos + int(Tqb[qq, b])))
                pos += int(Tqb[qq, b])
            qr.append((qs, pos))
        chunk_list.append((b0, b1, qr))
    T_total = pos

    shards = []
    for c in range(NC):
        m = owner == c
        sr, dl, eav, qq, bl = src_row[m], dst_loc[m], ea[m], q[m], blk[m]
        key = qq * NBLK + bl
        order = np.argsort(key, kind="stable")
        sr, dl, eav, qq, bl = sr[order], dl[order], eav[order], qq[order], bl[order]
        key = key[order]
        nslots = T_total * 128
        f_sr = np.zeros(nslots, np.int64)
        f_dl = np.zeros(nslots, np.int64)
        f_low = np.full(nslots, 30000.0, np.float32)
        f_ea = np.zeros(nslots, np.float32)
        # pad slots: need per-slot quartile base for valid idx; fill by group
        for qv in range(4):
            for b in range(NBLK):
                t0, n = tile_off[qv, b], int(Tqb[qv, b])
                f_sr[t0 * 128 : (t0 + n) * 128] = qv * QR
        cstart = np.concatenate([[0], np.cumsum(counts[c].reshape(-1))])
        posk = np.arange(len(key)) - cstart[key]
        slot = tile_off[qq, bl] * 128 + posk
        f_sr[slot] = sr
        f_dl[slot] = dl
        f_ea[slot] = eav
        f_low[slot] = (dl - 128 * bl).astype(np.float32)

        # per-gather-call wrapped int16 idx arrays
        sr16_parts, dl16_parts = [], []
        for (b0, b1, qr) in chunk_list:
            ch_t0, ch_t1 = qr[0][0], qr[3][1]
            for qv, (qs, qe) in enumerate(qr):
                fl = (f_sr[qs * 128 : qe * 128] - qv * QR).astype(np.int16)
                sr16_parts.append(_wrap16(fl, qe - qs))
            dl16_parts.append(_wrap16(f_dl[ch_t0 * 128 : ch_t1 * 128].astype(np.int16),
                                      ch_t1 - ch_t0))
        sr16 = np.concatenate(sr16_parts, 1)
        dl16 = np.concatenate(dl16_parts, 1)

        def wrap(a, dt):
            return np.ascontiguousarray(a.reshape(T_total, 128).T).astype(dt)

        shards.append(dict(
            sr16=np.ascontiguousarray(sr16), dl16=np.ascontiguousarray(dl16),
            low=wrap(f_low, np.float32), ea=wrap(f_ea, np.float32)))
    return shards, dict(Tqb=Tqb, chunk_list=chunk_list, blk_segs=blk_segs,
                        T_total=T_total)


def _ap(t: AP, dims) -> AP:
    """Rebuild an AP with explicit [step, count] dims (for 0-step broadcasts)."""
    return AP(t.tensor, t.offset, [list(d) for d in dims])


ALU = None  # set in _build_graph


MAX_GTILES = 8  # dma_gather per-call cap (ring limit ~1024-1536 idx)


def _build_graph(meta):
    QRC = NT // 4
    T_total = meta["T_total"]
    chunk_list = meta["chunk_list"]
    blk_segs = meta["blk_segs"]
    nc = bacc.Bacc(None, target_bir_lowering=False, debug=False)
    A = mybir.AluOpType
    ACTF = mybir.ActivationFunctionType

    # ---- DRAM I/O ----
    xT_d = nc.dram_tensor("xT", [128, NPAD], F32, kind="ExternalInput")
    sr16_d = nc.dram_tensor("sr16", [128, T_total * 8], mybir.dt.int16, kind="ExternalInput")
    dl16_d = nc.dram_tensor("dl16", [128, T_total * 8], mybir.dt.int16, kind="ExternalInput")
    low_d = nc.dram_tensor("low", [128, T_total], F32, kind="ExternalInput")
    ea_d = nc.dram_tensor("ea", [128, T_total], F32, kind="ExternalInput")
    iota_d = nc.dram_tensor("iota", [128, 128], BF16, kind="ExternalInput")
    ident_d = nc.dram_tensor("ident", [128, 128], F32, kind="ExternalInput")
    W1_d = nc.dram_tensor("W1", [128, 128], F32, kind="ExternalInput")
    as1_d = nc.dram_tensor("as1f", [1, 128], F32, kind="ExternalInput")
    ad1_d = nc.dram_tensor("ad1f", [1, 128], F32, kind="ExternalInput")
    We1_d = nc.dram_tensor("We1f", [1, 128], F32, kind="ExternalInput")
    ae1_d = nc.dram_tensor("ae1f", [1, 128], F32, kind="ExternalInput")
    b1_d = nc.dram_tensor("b1f", [1, 128], F32, kind="ExternalInput")
    W2_d = nc.dram_tensor("W2", [128, 64], F32, kind="ExternalInput")
    as2_d = nc.dram_tensor("as2f", [1, 64], F32, kind="ExternalInput")
    ad2_d = nc.dram_tensor("ad2f", [1, 64], F32, kind="ExternalInput")
    We2_d = nc.dram_tensor("We2f", [1, 64], F32, kind="ExternalInput")
    ae2_d = nc.dram_tensor("ae2f", [1, 64], F32, kind="ExternalInput")
    b2_d = nc.dram_tensor("b2f", [1, 64], F32, kind="ExternalInput")
    fcWT_d = nc.dram_tensor("fcWT", [1, 64], F32, kind="ExternalInput")
    fcb_d = nc.dram_tensor("fcb", [1, 1], F32, kind="ExternalInput")
    out_d = nc.dram_tensor("out", [NPAD, 1], F32, kind="ExternalOutput")

    # ---- internal DRAM ----
    t1loc = nc.dram_tensor("t1loc", [NPAD, 256], BF16)
    t1glob = nc.dram_tensor("t1glob", [NT, 256], BF16, addr_space="Shared")
    adst1g = nc.dram_tensor("adst1g", [NPAD, 128], BF16)
    t2loc = nc.dram_tensor("t2loc", [NPAD, 256], BF16)
    t2glob = nc.dram_tensor("t2glob", [NT, 256], BF16, addr_space="Shared")
    adst2g = nc.dram_tensor("adst2g", [NPAD, 128], BF16)


    with tile.TileContext(nc) as tc:
        with (
            tc.tile_pool(name="res", bufs=1) as res,
            tc.tile_pool(name="xin", bufs=3) as xin,
            tc.tile_pool(name="ps0", bufs=2, space="PSUM") as ps0,
            tc.tile_pool(name="row", bufs=3) as rowp,
            tc.tile_pool(name="g1", bufs=2) as g1p,
            tc.tile_pool(name="adp", bufs=2) as adp,
            tc.tile_pool(name="alp", bufs=2) as alp,
            tc.tile_pool(name="rhs", bufs=2) as rhsp,
            tc.tile_pool(name="pp", bufs=4) as pp,

            tc.tile_pool(name="epi", bufs=3) as epi,

        ):
            # ======== resident tiles ========
            iota_t = res.tile([128, 128], BF16, tag="iota")
            nc.sync.dma_start(iota_t[:], iota_d[:])
            ident_t = res.tile([128, 128], F32, tag="ident")
            nc.sync.dma_start(ident_t[:], ident_d[:])
            low_t = res.tile([128, T_total], F32, tag="low")
            nc.sync.dma_start(low_t[:], low_d[:])
            ea_t = res.tile([128, T_total], F32, tag="ea")
            nc.sync.dma_start(ea_t[:], ea_d[:])
            W1_t = res.tile([128, 128], F32, tag="W1")
            nc.sync.dma_start(W1_t[:], W1_d[:])
            W2_t = res.tile([128, 64], F32, tag="W2")
            nc.sync.dma_start(W2_t[:], W2_d[:])

            def load_row(d, n, tag):
                t = res.tile([1, n], F32, tag=tag)
                nc.sync.dma_start(t[:], d[:])
                return t

            as1_t = load_row(as1_d, 128, "as1")
            ad1_t = load_row(ad1_d, 128, "ad1")
            We1_t = load_row(We1_d, 128, "We1")
            ae1_t = load_row(ae1_d, 128, "ae1")
            b1_t = load_row(b1_d, 128, "b1")
            as2_t = load_row(as2_d, 64, "as2")
            ad2_t = load_row(ad2_d, 64, "ad2")
            We2_t = load_row(We2_d, 64, "We2")
            ae2_t = load_row(ae2_d, 64, "ae2")
            b2_t = load_row(b2_d, 64, "b2")
            fcWT_t = load_row(fcWT_d, 64, "fcWT")
            fcb_t = load_row(fcb_d, 1, "fcb")

            def pbc(src, n, tag):
                t = res.tile([128, n], F32, tag=tag)
                nc.gpsimd.partition_broadcast(t[:], src[:1, :n])
                return t

            as1_bc = pbc(as1_t, 128, "as1bc")
            ad1_bc = pbc(ad1_t, 128, "ad1bc")
            b1_bc = pbc(b1_t, 128, "b1bc")
            fcWT_bc = pbc(fcWT_t, 64, "fcWTbc")
            as2_bc = pbc(as2_t, 64, "as2bc")
            ad2_bc = pbc(ad2_t, 64, "ad2bc")

            # rhs0 = [W1 | A1s | A1d]  (A1s[k,h] = sum_c W1[k,32h+c]*as1[h,c])
            rhs0 = res.tile([128, 136], F32, tag="rhs0")
            nc.vector.tensor_copy(rhs0[:, :128], W1_t[:])
            ttr_scr = res.tile([128, 128], F32, tag="ttrscr")
            for h in range(H):
                sl = slice(32 * h, 32 * h + 32)
                nc.vector.tensor_tensor(ttr_scr[:, :32], W1_t[:, sl], as1_bc[:, sl], A.mult)
                nc.vector.tensor_reduce(rhs0[:, 128 + h : 129 + h], ttr_scr[:, :32],
                                        op=A.add, axis=mybir.AxisListType.X)
                nc.vector.tensor_tensor(ttr_scr[:, :32], W1_t[:, sl], ad1_bc[:, sl], A.mult)
                nc.vector.tensor_reduce(rhs0[:, 132 + h : 133 + h], ttr_scr[:, :32],
                                        op=A.add, axis=mybir.AxisListType.X)

            # we1 row -> [128,4] broadcast
            sc1 = res.tile([1, 128], F32, tag="sc1")
            nc.vector.tensor_tensor(sc1[:], We1_t[:], ae1_t[:], A.mult)
            we1_row = res.tile([1, 4], F32, tag="we1row")
            nc.vector.tensor_reduce(
                we1_row[:], sc1[:].rearrange("p (h c) -> p h c", h=4),
                op=A.add, axis=mybir.AxisListType.X)
            we1_t = res.tile([128, 4], F32, tag="we1t")
            nc.gpsimd.partition_broadcast(we1_t[:], we1_row[:])

            # we2 scalar, c scalar
            sc2 = res.tile([1, 64], F32, tag="sc2")
            nc.vector.tensor_tensor(sc2[:], We2_t[:], ae2_t[:], A.mult)
            we2_row = res.tile([1, 1], F32, tag="we2row")
            nc.vector.tensor_reduce(we2_row[:], sc2[:], op=A.add, axis=mybir.AxisListType.X)
            we2_bc = res.tile([128, 1], F32, tag="we2bc")
            nc.gpsimd.partition_broadcast(we2_bc[:], we2_row[:])
            sc3 = res.tile([1, 64], F32, tag="sc3")
            nc.vector.tensor_tensor(sc3[:], b2_t[:], fcWT_t[:], A.mult)
            c_row = res.tile([1, 1], F32, tag="crow")
            nc.vector.tensor_reduce(c_row[:], sc3[:], op=A.add, axis=mybir.AxisListType.X)
            nc.vector.tensor_tensor(c_row[:], c_row[:], fcb_t[:], A.add)
            c_bc = res.tile([128, 1], F32, tag="cbc")
            nc.gpsimd.partition_broadcast(c_bc[:], c_row[:])

            # v = W2@fcW, ws2 = W2@as2^T, wd2 = W2@ad2^T  (cols), then transpose+bcast
            def col_then_bcast(w_bc, tag):
                col = res.tile([128, 1], F32, tag=tag + "c")
                nc.vector.tensor_tensor(ttr_scr[:, :64], W2_t[:], w_bc[:], A.mult)
                nc.vector.tensor_reduce(col[:], ttr_scr[:, :64], op=A.add,
                                        axis=mybir.AxisListType.X)
                pst = ps0.tile([128, 128], F32, space="PSUM", tag="trps")
                nc.tensor.transpose(pst[:1, :], col[:], ident_t[:])
                rowt = res.tile([1, 128], F32, tag=tag + "r")
                nc.vector.tensor_copy(rowt[:], pst[:1, :])
                bc = res.tile([128, 128], F32, tag=tag + "b")
                nc.gpsimd.partition_broadcast(bc[:], rowt[:])
                return bc

            v_bc = col_then_bcast(fcWT_bc, "v")
            ws2_bc = col_then_bcast(as2_bc, "ws2")
            wd2_bc = col_then_bcast(ad2_bc, "wd2")

            # resident per-node arrays
            asrc1_sb = res.tile([128, NBLK, 4], F32, tag="asrc1sb")
            adst1_sb = res.tile([128, NBLK, 4], F32, tag="adst1sb")
            t2_sb = res.tile([128, NBLK, 2], F32, tag="t2sb")
            adst2_sb = res.tile([128, NBLK], F32, tag="adst2sb")
            loop_sb = res.tile([128, NBLK], F32, tag="loopsb")
            out_sb = res.tile([128, NBLK], F32, tag="outsb")
            if STAGE < 7:
                nc.vector.memset(out_sb[:], 0.0)
                nc.vector.memset(t2_sb[:], 0.0)
                nc.vector.memset(adst2_sb[:], 0.0)
                nc.vector.memset(loop_sb[:], 0.0)
                nc.vector.memset(asrc1_sb[:], 0.0)
                nc.vector.memset(adst1_sb[:], 0.0)

            def gather_split(out3, table, ixt, lo, hi, ob, ib):
                """dma_gather tiles [lo,hi): out3 col 0 = tile ob; ixt col 0 = tile ib."""
                t = lo
                while t < hi:
                    te = min(t + MAX_GTILES, hi)
                    nc.gpsimd.dma_gather(
                        out3[:, t - ob : te - ob, :], table,
                        ixt[:, 8 * (t - ib) : 8 * (te - ib)],
                        num_idxs=(te - t) * 128,
                        num_idxs_reg=(te - t) * 128,
                        elem_size=table.shape[1], single_packet=False)
                    t = te

            # ======== phase 0: node features ========
            for b in range(NBLK):
                xt = xin.tile([128, 128], F32, tag="xt")
                nc.sync.dma_start(xt[:], xT_d[:, 128 * b : 128 * b + 128])
                ps = ps0.tile([128, 136], F32, space="PSUM", tag="p0")
                nc.tensor.matmul(ps[:], xt[:], rhs0[:], start=True, stop=True)
                row = rowp.tile([128, 256], BF16, tag="row")
                nc.vector.tensor_copy(row[:, :128], ps[:, :128])
                nc.vector.tensor_copy(row[:, 128:132], ps[:, 128:132])
                nc.vector.memset(row[:, 132:], 0.0)
                nc.sync.dma_start(t1loc[128 * b : 128 * b + 128, :], row[:])
                nc.vector.tensor_copy(asrc1_sb[:, b, :], ps[:, 128:132])
                nc.vector.tensor_copy(adst1_sb[:, b, :], ps[:, 132:136])
                arow = rowp.tile([128, 128], BF16, tag="arow")
                nc.vector.tensor_copy(arow[:, 0:4], ps[:, 132:136])
                nc.vector.memset(arow[:, 4:], 0.0)
                nc.gpsimd.dma_start(adst1g[128 * b : 128 * b + 128, :], arow[:])
            if SIM_NOCOLL:
                nc.gpsimd.dma_start(t1glob[0:NPAD, :], t1loc[:, :])
            else:
                nc.gpsimd.collective_compute(
                    "AllGather", mybir.AluOpType.bypass,
                    replica_groups=[list(range(NC))],
                    ins=[t1loc.ap().opt()], outs=[t1glob.ap().opt()])

            # ======== L1 edge pass ========
            def lrelu(ap_io, scratch):
                nc.vector.tensor_scalar(scratch, ap_io, 0.2, None, A.mult)
                nc.vector.tensor_tensor(ap_io, ap_io, scratch, A.max)

            for (b0, b1, qr) in (chunk_list if STAGE >= 1 else []):
                t0, t1 = qr[0][0], qr[3][1]
                TB = t1 - t0
                g1 = g1p.tile([128, TB, 256], BF16, tag="g1")
                for qv, (qs, qe) in enumerate(qr):
                    if qe == qs:
                        continue
                    nt = qe - qs
                    ix = adp.tile([128, 8 * TB], mybir.dt.int16, tag="ix")
                    nc.sync.dma_start(ix[:, : 8 * nt], sr16_d[:, 8 * qs : 8 * qe])
                    if GATH & 1:
                        gather_split(g1, t1glob[qv * QRC : (qv + 1) * QRC, :],
                                     ix, qs, qe, t0, qs)
                ixd = adp.tile([128, 8 * TB], mybir.dt.int16, tag="ixd")
                nc.sync.dma_start(ixd[:], dl16_d[:, 8 * t0 : 8 * t1])
                ad = adp.tile([128, TB, 128], BF16, tag="ad")
                if GATH & 2:
                    gather_split(ad, adst1g[:, :], ixd, t0, t1, t0, t0)
                al = alp.tile([128, TB, 4], F32, tag="al")
                scr = alp.tile([128, TB, 4], F32, tag="scr")
                exb = alp.tile([128, TB, 4], BF16, tag="exb")
                if STAGE < 2:
                    continue
                nc.vector.tensor_copy(al[:], g1[:, :, 128:132])  # a_srcE (bf16->f32)
                nc.vector.tensor_copy(scr[:], ad[:, :, 0:4])
                nc.vector.tensor_tensor(al[:], al[:], scr[:], A.add)
                ea_sl = ea_t[:, t0:t1]
                ea_b = _ap(ea_sl, [ea_sl.ap[0], [ea_sl.ap[1][0], TB], [0, 4]])
                we1_b = _ap(we1_t[:, :], [we1_t[:, :].ap[0], [0, TB], [1, 4]])
                nc.vector.tensor_tensor(scr[:], ea_b, we1_b, A.mult)
                nc.vector.tensor_tensor(al[:], al[:], scr[:], A.add)
                lrelu(al[:], scr[:])
                nc.scalar.activation(al[:], al[:], ACTF.Exp)
                nc.vector.tensor_copy(exb[:], al[:])
                rhs = rhsp.tile([128, TB, 134], BF16, tag="rhs")
                if STAGE < 3:
                    continue
                exb_ap = exb[:, :, :]
                exb_b = _ap(exb_ap, list(exb_ap.ap) + [[0, 32]])
                nc.vector.tensor_tensor(
                    rhs[:, :, 0:128].rearrange("p t (h c) -> p t h c", h=4),
                    g1[:, :, 0:128].rearrange("p t (h c) -> p t h c", h=4),
                    exb_b, A.mult)
                nc.vector.tensor_copy(rhs[:, :, 128:132], exb[:])
                nc.vector.tensor_copy(rhs[:, :, 132:133],
                                      _ap(ea_sl, [ea_sl.ap[0], [ea_sl.ap[1][0], TB], [0, 1]]))
                nc.vector.memset(rhs[:, :, 133:134], 1.0)

                pall = epi.tile([128, b1 - b0, 134], F32, tag="pall")
                for b in (range(b0, b1) if STAGE >= 4 else []):
                    pB = ps0.tile([128, 134], F32, space="PSUM", tag="pB")
                    tiles = [t for (ts, te) in blk_segs[b] for t in range(ts, te)]
                    assert tiles, f"block {b} has no tiles"
                    for i, t in enumerate(tiles):
                        P = pp.tile([128, 128], BF16, tag="P")
                        nc.vector.tensor_scalar(
                            P[:], iota_t[:], low_t[:, t : t + 1], None, A.is_equal)
                        nc.tensor.matmul(
                            pB[:], P[:], rhs[:, t - t0, :],
                            start=(i == 0), stop=(i == len(tiles) - 1))

                    # ---- stash psum for batched epilogue ----
                    nc.vector.tensor_copy(
                        pall[:, b - b0, :], pB[:])

                # ---- batched L1 epilogue for the chunk ----
                if STAGE < 4:
                    continue
                NB = b1 - b0
                den = pall[:, :, 128:132]
                degc = epi.tile([128, NB, 1], F32, tag="degc")
                nc.vector.tensor_scalar(degc[:], pall[:, :, 133:134], 1.0, None, A.max)
                nc.vector.reciprocal(degc[:], degc[:])
                la2d = loop_sb[:, b0:b1]
                la3 = _ap(la2d, list(la2d.ap) + [[1, 1]])
                nc.vector.tensor_tensor(la3, pall[:, :, 132:133], degc[:], A.mult)
                alc = epi.tile([128, NB, 4], F32, tag="alc")
                scr4 = epi.tile([128, NB, 4], F32, tag="scr4")
                nc.vector.tensor_tensor(alc[:], asrc1_sb[:, b0:b1, :], adst1_sb[:, b0:b1, :], A.add)
                la_b = _ap(la3, [la3.ap[0], la3.ap[1], [0, 4]])
                we1_b3 = _ap(we1_t[:, :], [we1_t[:, :].ap[0], [0, NB], [1, 4]])
                nc.vector.tensor_tensor(scr4[:], la_b, we1_b3, A.mult)
                nc.vector.tensor_tensor(alc[:], alc[:], scr4[:], A.add)
                lrelu(alc[:], scr4[:])
                nc.scalar.activation(alc[:], alc[:], ACTF.Exp)
                dent = epi.tile([128, NB, 4], F32, tag="dent")
                nc.vector.tensor_tensor(dent[:], den, alc[:], A.add)
                rec = epi.tile([128, NB, 4], F32, tag="rec")
                nc.vector.reciprocal(rec[:], dent[:])
                exlb = epi.tile([128, NB, 4], BF16, tag="exlb")
                nc.vector.tensor_copy(exlb[:], alc[:])
                hb = epi.tile([128, NB, 128], BF16, tag="hb")
                nc.sync.dma_start(
                    hb[:], t1loc[128 * b0 : 128 * b1, 0:128].rearrange(
                        "(a p) c -> p a c", p=128))
                num = epi.tile([128, NB, 128], F32, tag="num")
                exlb_b = _ap(exlb[:, :, :], list(exlb[:, :, :].ap) + [[0, 32]])
                nc.vector.tensor_tensor(
                    num[:].rearrange("p b (h c) -> p b h c", h=4),
                    hb[:].rearrange("p b (h c) -> p b h c", h=4), exlb_b, A.mult)
                nc.vector.tensor_tensor(num[:], num[:], pall[:, :, 0:128], A.add)
                rec_b = _ap(rec[:, :, :], list(rec[:, :, :].ap) + [[0, 32]])
                o1 = epi.tile([128, NB, 128], F32, tag="o1")
                nc.vector.tensor_tensor(
                    o1[:].rearrange("p b (h c) -> p b h c", h=4),
                    num[:].rearrange("p b (h c) -> p b h c", h=4), rec_b, A.mult)
                b1_b3 = _ap(b1_bc[:, :], [b1_bc[:, :].ap[0], [0, NB], [1, 128]])
                nc.vector.tensor_tensor(o1[:], o1[:], b1_b3, A.add)
                mn = epi.tile([128, NB, 128], F32, tag="mn")
                nc.vector.tensor_scalar(mn[:], o1[:], 0.0, None, A.min)
                nc.scalar.activation(mn[:], mn[:], ACTF.Exp)
                g = epi.tile([128, NB, 128], F32, tag="g")
                nc.vector.tensor_scalar(g[:], o1[:], 0.0, None, A.max)
                nc.vector.tensor_tensor(g[:], g[:], mn[:], A.add)
                nc.vector.tensor_scalar(g[:], g[:], -1.0, None, A.add)
                gscr = epi.tile([128, NB, 128], F32, tag="gscr")
                v_b3 = _ap(v_bc[:, :], [v_bc[:, :].ap[0], [0, NB], [1, 128]])
                ws2_b3 = _ap(ws2_bc[:, :], [ws2_bc[:, :].ap[0], [0, NB], [1, 128]])
                wd2_b3 = _ap(wd2_bc[:, :], [wd2_bc[:, :].ap[0], [0, NB], [1, 128]])
                nc.vector.tensor_tensor(gscr[:], g[:], v_b3, A.mult)
                nc.vector.tensor_reduce(
                    t2_sb[:, b0:b1, 0:1], gscr[:], op=A.add, axis=mybir.AxisListType.X)
                nc.vector.tensor_tensor(gscr[:], g[:], ws2_b3, A.mult)
                nc.vector.tensor_reduce(
                    t2_sb[:, b0:b1, 1:2], gscr[:], op=A.add, axis=mybir.AxisListType.X)
                nc.vector.tensor_tensor(gscr[:], g[:], wd2_b3, A.mult)
                ad2d = adst2_sb[:, b0:b1]
                nc.vector.tensor_reduce(
                    _ap(ad2d, list(ad2d.ap) + [[1, 1]]), gscr[:],
                    op=A.add, axis=mybir.AxisListType.X)
                t2row = epi.tile([128, NB, 256], BF16, tag="t2row")
                nc.vector.memset(t2row[:], 0.0)
                nc.vector.tensor_copy(t2row[:, :, 0:2], t2_sb[:, b0:b1, :])
                nc.sync.dma_start(
                    t2loc[128 * b0 : 128 * b1, :].rearrange("(a p) c -> p a c", p=128),
                    t2row[:])
                a2row = epi.tile([128, NB, 128], BF16, tag="a2row")
                nc.vector.memset(a2row[:], 0.0)
                ad2d2 = adst2_sb[:, b0:b1]
                nc.vector.tensor_copy(
                    a2row[:, :, 0:1], _ap(ad2d2, list(ad2d2.ap) + [[1, 1]]))
                nc.sync.dma_start(
                    adst2g[128 * b0 : 128 * b1, :].rearrange("(a p) c -> p a c", p=128),
                    a2row[:])

            # ======== layer 2 ========
            if SIM_NOCOLL:
                nc.gpsimd.dma_start(t2glob[0:NPAD, :], t2loc[:, :])
            else:
                nc.gpsimd.collective_compute(
                    "AllGather", mybir.AluOpType.bypass,
                    replica_groups=[list(range(NC))],
                    ins=[t2loc.ap().opt()], outs=[t2glob.ap().opt()])

            for (b0, b1, qr) in (chunk_list if STAGE >= 5 else []):
                t0, t1 = qr[0][0], qr[3][1]
                TB = t1 - t0
                pr = g1p.tile([128, TB, 256], BF16, tag="g1")
                for qv, (qs, qe) in enumerate(qr):
                    if qe == qs:
                        continue
                    nt = qe - qs
                    ix2 = adp.tile([128, 8 * TB], mybir.dt.int16, tag="ix")
                    nc.sync.dma_start(ix2[:, : 8 * nt], sr16_d[:, 8 * qs : 8 * qe])
                    gather_split(pr, t2glob[qv * QRC : (qv + 1) * QRC, :],
                                 ix2, qs, qe, t0, qs)
                ixd2 = adp.tile([128, 8 * TB], mybir.dt.int16, tag="ixd")
                nc.sync.dma_start(ixd2[:], dl16_d[:, 8 * t0 : 8 * t1])
                ad2 = adp.tile([128, TB, 128], BF16, tag="ad")
                gather_split(ad2, adst2g[:, :], ixd2, t0, t1, t0, t0)
                al2 = alp.tile([128, TB], F32, tag="al")
                scr2 = alp.tile([128, TB], F32, tag="scr")
                if STAGE < 6:
                    continue
                nc.vector.tensor_copy(al2[:], pr[:, :, 1])
                nc.vector.tensor_copy(scr2[:], ad2[:, :, 0])
                nc.vector.tensor_tensor(al2[:], al2[:], scr2[:], A.add)
                nc.vector.tensor_scalar(scr2[:], ea_t[:, t0:t1], we2_bc[:, :1], None, A.mult)
                nc.vector.tensor_tensor(al2[:], al2[:], scr2[:], A.add)
                lrelu(al2[:], scr2[:])
                nc.scalar.activation(al2[:], al2[:], ACTF.Exp)
                rhs2 = rhsp.tile([128, TB, 2], BF16, tag="rhs")
                nc.vector.tensor_copy(rhs2[:, :, 0:1], pr[:, :, 0:1])
                nc.vector.memset(rhs2[:, :, 1:2], 1.0)

                p2all = epi.tile([128, b1 - b0, 2], F32, tag="p2all")
                if STAGE < 7:
                    continue
                for b in range(b0, b1):
                    pB2 = ps0.tile([128, 2], F32, space="PSUM", tag="pB2")
                    tiles = [t for (ts, te) in blk_segs[b] for t in range(ts, te)]
                    for i, t in enumerate(tiles):
                        P2 = pp.tile([128, 128], BF16, tag="P")
                        nc.vector.tensor_scalar(
                            P2[:], iota_t[:], low_t[:, t : t + 1],
                            al2[:, t - t0 : t - t0 + 1], A.is_equal, A.mult)
                        nc.tensor.matmul(
                            pB2[:], P2[:], rhs2[:, t - t0, :],
                            start=(i == 0), stop=(i == len(tiles) - 1))

                    nc.vector.tensor_copy(p2all[:, b - b0, :], pB2[:])

                # ---- batched L2 epilogue ----
                NB = b1 - b0
                al1 = epi.tile([128, NB, 1], F32, tag="al1")
                s1 = epi.tile([128, NB, 1], F32, tag="s1")
                nc.vector.tensor_tensor(
                    al1[:], t2_sb[:, b0:b1, 1:2],
                    _ap(adst2_sb[:, b0:b1], list(adst2_sb[:, b0:b1].ap) + [[1, 1]]), A.add)
                we2_b3 = _ap(we2_bc[:, :], [we2_bc[:, :].ap[0], [0, NB], [1, 1]])
                nc.vector.tensor_tensor(
                    s1[:], _ap(loop_sb[:, b0:b1], list(loop_sb[:, b0:b1].ap) + [[1, 1]]),
                    we2_b3, A.mult)
                nc.vector.tensor_tensor(al1[:], al1[:], s1[:], A.add)
                lrelu(al1[:], s1[:])
                nc.scalar.activation(al1[:], al1[:], ACTF.Exp)
                numt = epi.tile([128, NB, 1], F32, tag="numt")
                nc.vector.tensor_tensor(numt[:], al1[:], t2_sb[:, b0:b1, 0:1], A.mult)
                nc.vector.tensor_tensor(numt[:], numt[:], p2all[:, :, 0:1], A.add)
                dent2 = epi.tile([128, NB, 1], F32, tag="dent2")
                nc.vector.tensor_tensor(dent2[:], p2all[:, :, 1:2], al1[:], A.add)
                nc.vector.reciprocal(dent2[:], dent2[:])
                fin = _ap(out_sb[:, b0:b1], list(out_sb[:, b0:b1].ap) + [[1, 1]])
                nc.vector.tensor_tensor(fin, numt[:], dent2[:], A.mult)
                c_b3 = _ap(c_bc[:, :], [c_bc[:, :].ap[0], [0, NB], [1, 1]])
                nc.vector.tensor_tensor(fin, fin, c_b3, A.add)

            with nc.allow_non_contiguous_dma(reason="block-major output"):
                nc.sync.dma_start(
                    out_d[:, 0].rearrange("(b p) -> p b", p=128), out_sb[:])

    nc.compile()
    return nc


def _make_in_maps(inputs, shards):
    x = np.asarray(inputs["x"], np.float32)
    xTp = np.zeros((NC, 128, NPAD), np.float32)
    for c in range(NC):
        xTp[c, :, :NLOC] = x[c * NLOC : (c + 1) * NLOC].T
    iota = np.tile(np.arange(128, dtype=np.float32), (128, 1)).astype(BF)
    ident = np.eye(128, dtype=np.float32)
    common = dict(
        iota=iota, ident=ident,
        W1=np.asarray(inputs["W1"], np.float32),
        as1f=np.asarray(inputs["as1"], np.float32).reshape(1, 128),
        ad1f=np.asarray(inputs["ad1"], np.float32).reshape(1, 128),
        We1f=np.asarray(inputs["We1"], np.float32).reshape(1, 128),
        ae1f=np.asarray(inputs["ae1"], np.float32).reshape(1, 128),
        b1f=np.asarray(inputs["b1"], np.float32).reshape(1, 128),
        W2=np.asarray(inputs["W2"], np.float32),
        as2f=np.asarray(inputs["as2"], np.float32).reshape(1, 64),
        ad2f=np.asarray(inputs["ad2"], np.float32).reshape(1, 64),
        We2f=np.asarray(inputs["We2"], np.float32).reshape(1, 64),
        ae2f=np.asarray(inputs["ae2"], np.float32).reshape(1, 64),
        b2f=np.asarray(inputs["b2"], np.float32).reshape(1, 64),
        fcWT=np.asarray(inputs["fcW"], np.float32).reshape(1, 64),
        fcb=np.asarray(inputs["fcb"], np.float32).reshape(1, 1),
    )
    return [dict(common, xT=xTp[c], sr16=shards[c]["sr16"], dl16=shards[c]["dl16"],
                 low=shards[c]["low"], ea=shards[c]["ea"]) for c in range(NC)]


def kernel(x, edge_index, edge_attr, W1, as1, ad1, We1, ae1, b1,
           W2, as2, ad2, We2, ae2, b2, fcW, fcb):
    shards, meta = _build_shards(np.asarray(edge_index), np.asarray(edge_attr))
    nc = _build_graph(meta)
    in_maps = _make_in_maps(
        dict(x=x, W1=W1, as1=as1, ad1=ad1, We1=We1, ae1=ae1, b1=b1, W2=W2,
             as2=as2, ad2=ad2, We2=We2, ae2=ae2, b2=b2, fcW=fcW, fcb=fcb),
        shards)
    res = run_bass_kernel_spmd(nc, in_maps, core_ids=list(range(NC)))
    out = np.concatenate(
        [res.results[c]["out"][:NLOC, :] for c in range(NC)], axis=0)
    return out.astype(np.float32)



# revision 3
# speedup vs baseline: 1.3283x; 1.3283x over previous
# GAT 2-layer message-passing kernel for 8 TRN2 NeuronCores.
# Strategy: destination-node partitioning (12500 dst nodes/core), edges
# sharded by dst owner and grouped into 128-edge tiles per 128-node dst
# block. Scatter/segment-softmax via one-hot matmuls accumulating in PSUM.
# Node features gathered per edge via indirect DMA from an AllGathered
# bf16 table. All floating-point math runs on device.
import sys

sys.path.insert(0, "/opt/trn_rl_repo")

import numpy as np
import ml_dtypes

import concourse.bass as bass
import concourse.bacc as bacc
import concourse.mybir as mybir
import concourse.tile as tile
from concourse.bass import AP, IndirectOffsetOnAxis
from concourse.bass_utils import run_bass_kernel_spmd

F32 = mybir.dt.float32
BF16 = mybir.dt.bfloat16
I32 = mybir.dt.int32
BF = ml_dtypes.bfloat16

N, E, NC = 100000, 1600000, 8
F, H, C1, C2 = 128, 4, 32, 64
NLOC = N // NC            # 12500 dst nodes per core
NPAD = 12544              # 98 * 128
NBLK = NPAD // 128        # 98
NT = NC * NPAD            # global table rows
CHUNK_BLKS = 2            # dst blocks per gather chunk
STAGE = 7                 # build stages (debug bisect)
GATH = 3                  # bitmask: 1=sr gathers, 2=dl gathers
SIM_NOCOLL = False        # replace collectives with local DMAs (TimelineSim)


QR = None  # set below: NT // 4


def _wrap16(flat_i16, ntiles):
    """dma_gather idx layout: logical idx j -> partition j%16, col j//16,
    replicated to all 8 16-partition groups. flat [ntiles*128] -> [128, ntiles*8]."""
    S = ntiles * 8
    a = np.zeros((16, S), np.int16)
    j = np.arange(ntiles * 128)
    a[j % 16, j // 16] = flat_i16
    return np.tile(a, (8, 1))


def _build_shards(edge_index, edge_attr):
    """Shard edges by dst owner, group by (chunk, src-quartile, dst-block),
    pad each (q, b) group to shared tile counts."""
    global QR
    QR = NT // 4
    src = edge_index[0].astype(np.int64)
    dst = edge_index[1].astype(np.int64)
    ea = edge_attr.astype(np.float32)
    owner = dst // NLOC
    dst_loc = dst % NLOC
    src_row = (src // NLOC) * NPAD + (src % NLOC)
    q = src_row // QR
    blk = dst_loc // 128

    counts = np.zeros((NC, 4, NBLK), np.int64)
    np.add.at(counts, (owner, q, blk), 1)
    Tqb = -(-counts.max(0) // 128)  # [4, NBLK] tiles per (q, b)

    # slot ordering: for chunk: for q: for b in chunk: Tqb[q][b] tiles
    nch = -(-NBLK // CHUNK_BLKS)
    tile_off = np.zeros((4, NBLK), np.int64)  # tile offset of each (q, b)
    call_ranges = []  # per chunk: list of 4 (tile_start, tile_end) per q
    blk_segs = [[] for _ in range(NBLK)]  # per block: [(tstart, tend), ...]
    pos = 0
    chunk_list = []
    for c in range(nch):
        b0, b1 = c * CHUNK_BLKS, min((c + 1) * CHUNK_BLKS, NBLK)
        qr = []
        for qq in range(4):
            qs = pos
            for b in range(b0, b1):
                tile_off[qq, b] = pos
                blk_segs[b].append((pos, pos + int(Tqb[qq, b])))
                pos += int(Tqb[qq, b])
            qr.append((qs, pos))
        chunk_list.append((b0, b1, qr))
    T_total = pos

    shards = []
    for c in range(NC):
        m = owner == c
        sr, dl, eav, qq, bl = src_row[m], dst_loc[m], ea[m], q[m], blk[m]
        key = qq * NBLK + bl
        order = np.argsort(key, kind="stable")
        sr, dl, eav, qq, bl = sr[order], dl[order], eav[order], qq[order], bl[order]
        key = key[order]
        nslots = T_total * 128
        f_sr = np.zeros(nslots, np.int64)
        f_dl = np.zeros(nslots, np.int64)
        f_low = np.full(nslots, 30000.0, np.float32)
        f_ea = np.zeros(nslots, np.float32)
        # pad slots: need per-slot quartile base for valid idx; fill by group
        for qv in range(4):
            for b in range(NBLK):
                t0, n = tile_off[qv, b], int(Tqb[qv, b])
                f_sr[t0 * 128 : (t0 + n) * 128] = qv * QR
        cstart = np.concatenate([[0], np.cumsum(counts[c].reshape(-1))])
        posk = np.arange(len(key)) - cstart[key]
        slot = tile_off[qq, bl] * 128 + posk
        f_sr[slot] = sr
        f_dl[slot] = dl
        f_ea[slot] = eav
        f_low[slot] = (dl - 128 * bl).astype(np.float32)

        # per-gather-call wrapped int16 idx arrays
        sr16_parts, dl16_parts = [], []
        for (b0, b1, qr) in chunk_list:
            ch_t0, ch_t1 = qr[0][0], qr[3][1]
            for qv, (qs, qe) in enumerate(qr):
                fl = (f_sr[qs * 128 : qe * 128] - qv * QR).astype(np.int16)
                sr16_parts.append(_wrap16(fl, qe - qs))
            dl16_parts.append(_wrap16(f_dl[ch_t0 * 128 : ch_t1 * 128].astype(np.int16),
                                      ch_t1 - ch_t0))
        sr16 = np.concatenate(sr16_parts, 1)
        dl16 = np.concatenate(dl16_parts, 1)

        def wrap(a, dt):
            return np.ascontiguousarray(a.reshape(T_total, 128).T).astype(dt)

        shards.append(dict(
            sr16=np.ascontiguousarray(sr16), dl16=np.ascontiguousarray(dl16),
            low=wrap(f_low, np.float32), ea=wrap(f_ea, np.float32)))
    return shards, dict(Tqb=Tqb, chunk_list=chunk_list, blk_segs=blk_segs,
                        T_total=T_total)


def _ap(t: AP, dims) -> AP:
    """Rebuild an AP with explicit [step, count] dims (for 0-step broadcasts)."""
    return AP(t.tensor, t.offset, [list(d) for d in dims])


ALU = None  # set in _build_graph


MAX_GTILES = 8  # dma_gather per-call cap (ring limit ~1024-1536 idx)


def _build_graph(meta):
    QRC = NT // 4
    T_total = meta["T_total"]
    chunk_list = meta["chunk_list"]
    blk_segs = meta["blk_segs"]
    nc = bacc.Bacc(None, target_bir_lowering=False, debug=False)
    A = mybir.AluOpType
    ACTF = mybir.ActivationFunctionType

    # ---- DRAM I/O ----
    xT_d = nc.dram_tensor("xT", [128, NPAD], F32, kind="ExternalInput")
    sr16_d = nc.dram_tensor("sr16", [128, T_total * 8], mybir.dt.int16, kind="ExternalInput")
    dl16_d = nc.dram_tensor("dl16", [128, T_total * 8], mybir.dt.int16, kind="ExternalInput")
    low_d = nc.dram_tensor("low", [128, T_total], F32, kind="ExternalInput")
    ea_d = nc.dram_tensor("ea", [128, T_total], F32, kind="ExternalInput")
    iota_d = nc.dram_tensor("iota", [128, 128], BF16, kind="ExternalInput")
    ident_d = nc.dram_tensor("ident", [128, 128], F32, kind="ExternalInput")
    W1_d = nc.dram_tensor("W1", [128, 128], F32, kind="ExternalInput")
    as1_d = nc.dram_tensor("as1f", [1, 128], F32, kind="ExternalInput")
    ad1_d = nc.dram_tensor("ad1f", [1, 128], F32, kind="ExternalInput")
    We1_d = nc.dram_tensor("We1f", [1, 128], F32, kind="ExternalInput")
    ae1_d = nc.dram_tensor("ae1f", [1, 128], F32, kind="ExternalInput")
    b1_d = nc.dram_tensor("b1f", [1, 128], F32, kind="ExternalInput")
    W2_d = nc.dram_tensor("W2", [128, 64], F32, kind="ExternalInput")
    as2_d = nc.dram_tensor("as2f", [1, 64], F32, kind="ExternalInput")
    ad2_d = nc.dram_tensor("ad2f", [1, 64], F32, kind="ExternalInput")
    We2_d = nc.dram_tensor("We2f", [1, 64], F32, kind="ExternalInput")
    ae2_d = nc.dram_tensor("ae2f", [1, 64], F32, kind="ExternalInput")
    b2_d = nc.dram_tensor("b2f", [1, 64], F32, kind="ExternalInput")
    fcWT_d = nc.dram_tensor("fcWT", [1, 64], F32, kind="ExternalInput")
    fcb_d = nc.dram_tensor("fcb", [1, 1], F32, kind="ExternalInput")
    out_d = nc.dram_tensor("out", [NPAD, 1], F32, kind="ExternalOutput")

    # ---- internal DRAM ----
    t1loc = nc.dram_tensor("t1loc", [NPAD, 256], BF16)
    t1glob = nc.dram_tensor("t1glob", [NT, 256], BF16, addr_space="Shared")
    adst1g = nc.dram_tensor("adst1g", [NPAD, 128], BF16)
    t2loc = nc.dram_tensor("t2loc", [NPAD, 128], BF16)
    t2glob = nc.dram_tensor("t2glob", [NT, 128], BF16, addr_space="Shared")
    adst2g = nc.dram_tensor("adst2g", [NPAD, 128], BF16)


    with tile.TileContext(nc) as tc:
        with (
            tc.tile_pool(name="res", bufs=1) as res,
            tc.tile_pool(name="xin", bufs=3) as xin,
            tc.tile_pool(name="ps0", bufs=2, space="PSUM") as ps0,
            tc.tile_pool(name="row", bufs=3) as rowp,
            tc.tile_pool(name="g1", bufs=3) as g1p,
            tc.tile_pool(name="adp", bufs=3) as adp,
            tc.tile_pool(name="alp", bufs=2) as alp,
            tc.tile_pool(name="rhs", bufs=2) as rhsp,
            tc.tile_pool(name="pp", bufs=4) as pp,

            tc.tile_pool(name="epi", bufs=3) as epi,

        ):
            # ======== resident tiles ========
            iota_t = res.tile([128, 128], BF16, tag="iota")
            nc.sync.dma_start(iota_t[:], iota_d[:])
            ident_t = res.tile([128, 128], F32, tag="ident")
            nc.sync.dma_start(ident_t[:], ident_d[:])
            low_t = res.tile([128, T_total], F32, tag="low")
            nc.sync.dma_start(low_t[:], low_d[:])
            ea_t = res.tile([128, T_total], F32, tag="ea")
            nc.sync.dma_start(ea_t[:], ea_d[:])
            W1_t = res.tile([128, 128], F32, tag="W1")
            nc.sync.dma_start(W1_t[:], W1_d[:])
            W2_t = res.tile([128, 64], F32, tag="W2")
            nc.sync.dma_start(W2_t[:], W2_d[:])

            def load_row(d, n, tag):
                t = res.tile([1, n], F32, tag=tag)
                nc.sync.dma_start(t[:], d[:])
                return t

            as1_t = load_row(as1_d, 128, "as1")
            ad1_t = load_row(ad1_d, 128, "ad1")
            We1_t = load_row(We1_d, 128, "We1")
            ae1_t = load_row(ae1_d, 128, "ae1")
            b1_t = load_row(b1_d, 128, "b1")
            as2_t = load_row(as2_d, 64, "as2")
            ad2_t = load_row(ad2_d, 64, "ad2")
            We2_t = load_row(We2_d, 64, "We2")
            ae2_t = load_row(ae2_d, 64, "ae2")
            b2_t = load_row(b2_d, 64, "b2")
            fcWT_t = load_row(fcWT_d, 64, "fcWT")
            fcb_t = load_row(fcb_d, 1, "fcb")

            def pbc(src, n, tag):
                t = res.tile([128, n], F32, tag=tag)
                nc.gpsimd.partition_broadcast(t[:], src[:1, :n])
                return t

            as1_bc = pbc(as1_t, 128, "as1bc")
            ad1_bc = pbc(ad1_t, 128, "ad1bc")
            b1_bc = pbc(b1_t, 128, "b1bc")
            fcWT_bc = pbc(fcWT_t, 64, "fcWTbc")
            as2_bc = pbc(as2_t, 64, "as2bc")
            ad2_bc = pbc(ad2_t, 64, "ad2bc")

            # rhs0 = [W1 | A1s | A1d]  (A1s[k,h] = sum_c W1[k,32h+c]*as1[h,c])
            rhs0 = res.tile([128, 136], F32, tag="rhs0")
            nc.vector.tensor_copy(rhs0[:, :128], W1_t[:])
            ttr_scr = res.tile([128, 128], F32, tag="ttrscr")
            for h in range(H):
                sl = slice(32 * h, 32 * h + 32)
                nc.vector.tensor_tensor(ttr_scr[:, :32], W1_t[:, sl], as1_bc[:, sl], A.mult)
                nc.vector.tensor_reduce(rhs0[:, 128 + h : 129 + h], ttr_scr[:, :32],
                                        op=A.add, axis=mybir.AxisListType.X)
                nc.vector.tensor_tensor(ttr_scr[:, :32], W1_t[:, sl], ad1_bc[:, sl], A.mult)
                nc.vector.tensor_reduce(rhs0[:, 132 + h : 133 + h], ttr_scr[:, :32],
                                        op=A.add, axis=mybir.AxisListType.X)

            # we1 row -> [128,4] broadcast
            sc1 = res.tile([1, 128], F32, tag="sc1")
            nc.vector.tensor_tensor(sc1[:], We1_t[:], ae1_t[:], A.mult)
            we1_row = res.tile([1, 4], F32, tag="we1row")
            nc.vector.tensor_reduce(
                we1_row[:], sc1[:].rearrange("p (h c) -> p h c", h=4),
                op=A.add, axis=mybir.AxisListType.X)
            we1_t = res.tile([128, 4], F32, tag="we1t")
            nc.gpsimd.partition_broadcast(we1_t[:], we1_row[:])

            # we2 scalar, c scalar
            sc2 = res.tile([1, 64], F32, tag="sc2")
            nc.vector.tensor_tensor(sc2[:], We2_t[:], ae2_t[:], A.mult)
            we2_row = res.tile([1, 1], F32, tag="we2row")
            nc.vector.tensor_reduce(we2_row[:], sc2[:], op=A.add, axis=mybir.AxisListType.X)
            we2_bc = res.tile([128, 1], F32, tag="we2bc")
            nc.gpsimd.partition_broadcast(we2_bc[:], we2_row[:])
            sc3 = res.tile([1, 64], F32, tag="sc3")
            nc.vector.tensor_tensor(sc3[:], b2_t[:], fcWT_t[:], A.mult)
            c_row = res.tile([1, 1], F32, tag="crow")
            nc.vector.tensor_reduce(c_row[:], sc3[:], op=A.add, axis=mybir.AxisListType.X)
            nc.vector.tensor_tensor(c_row[:], c_row[:], fcb_t[:], A.add)
            c_bc = res.tile([128, 1], F32, tag="cbc")
            nc.gpsimd.partition_broadcast(c_bc[:], c_row[:])

            # v = W2@fcW, ws2 = W2@as2^T, wd2 = W2@ad2^T  (cols), then transpose+bcast
            def col_then_bcast(w_bc, tag):
                col = res.tile([128, 1], F32, tag=tag + "c")
                nc.vector.tensor_tensor(ttr_scr[:, :64], W2_t[:], w_bc[:], A.mult)
                nc.vector.tensor_reduce(col[:], ttr_scr[:, :64], op=A.add,
                                        axis=mybir.AxisListType.X)
                pst = ps0.tile([128, 128], F32, space="PSUM", tag="trps")
                nc.tensor.transpose(pst[:1, :], col[:], ident_t[:])
                rowt = res.tile([1, 128], F32, tag=tag + "r")
                nc.vector.tensor_copy(rowt[:], pst[:1, :])
                bc = res.tile([128, 128], F32, tag=tag + "b")
                nc.gpsimd.partition_broadcast(bc[:], rowt[:])
                return bc

            v_bc = col_then_bcast(fcWT_bc, "v")
            ws2_bc = col_then_bcast(as2_bc, "ws2")
            wd2_bc = col_then_bcast(ad2_bc, "wd2")

            # resident per-node arrays
            asrc1_sb = res.tile([128, NBLK, 4], F32, tag="asrc1sb")
            adst1_sb = res.tile([128, NBLK, 4], F32, tag="adst1sb")
            t2_sb = res.tile([128, NBLK, 2], F32, tag="t2sb")
            adst2_sb = res.tile([128, NBLK], F32, tag="adst2sb")
            loop_sb = res.tile([128, NBLK], F32, tag="loopsb")
            out_sb = res.tile([128, NBLK], F32, tag="outsb")
            if STAGE < 7:
                nc.vector.memset(out_sb[:], 0.0)
                nc.vector.memset(t2_sb[:], 0.0)
                nc.vector.memset(adst2_sb[:], 0.0)
                nc.vector.memset(loop_sb[:], 0.0)
                nc.vector.memset(asrc1_sb[:], 0.0)
                nc.vector.memset(adst1_sb[:], 0.0)

            def gather_split(out3, table, ixt, lo, hi, ob, ib):
                """dma_gather tiles [lo,hi): out3 col 0 = tile ob; ixt col 0 = tile ib."""
                t = lo
                while t < hi:
                    te = min(t + MAX_GTILES, hi)
                    nc.gpsimd.dma_gather(
                        out3[:, t - ob : te - ob, :], table,
                        ixt[:, 8 * (t - ib) : 8 * (te - ib)],
                        num_idxs=(te - t) * 128,
                        num_idxs_reg=(te - t) * 128,
                        elem_size=table.shape[1], single_packet=False)
                    t = te

            # ======== phase 0: node features ========
            for b in range(NBLK):
                xt = xin.tile([128, 128], F32, tag="xt")
                nc.sync.dma_start(xt[:], xT_d[:, 128 * b : 128 * b + 128])
                ps = ps0.tile([128, 136], F32, space="PSUM", tag="p0")
                nc.tensor.matmul(ps[:], xt[:], rhs0[:], start=True, stop=True)
                row = rowp.tile([128, 256], BF16, tag="row")
                nc.vector.tensor_copy(row[:, :128], ps[:, :128])
                nc.vector.tensor_copy(row[:, 128:132], ps[:, 128:132])
                nc.vector.memset(row[:, 132:], 0.0)
                nc.sync.dma_start(t1loc[128 * b : 128 * b + 128, :], row[:])
                nc.vector.tensor_copy(asrc1_sb[:, b, :], ps[:, 128:132])
                nc.vector.tensor_copy(adst1_sb[:, b, :], ps[:, 132:136])
                arow = rowp.tile([128, 128], BF16, tag="arow")
                nc.vector.tensor_copy(arow[:, 0:4], ps[:, 132:136])
                nc.vector.memset(arow[:, 4:], 0.0)
                nc.gpsimd.dma_start(adst1g[128 * b : 128 * b + 128, :], arow[:])
            if SIM_NOCOLL:
                nc.gpsimd.dma_start(t1glob[0:NPAD, :], t1loc[:, :])
            else:
                nc.gpsimd.collective_compute(
                    "AllGather", mybir.AluOpType.bypass,
                    replica_groups=[list(range(NC))],
                    ins=[t1loc.ap().opt()], outs=[t1glob.ap().opt()])

            # ======== L1 edge pass ========
            def lrelu(ap_io, scratch):
                nc.vector.tensor_scalar(scratch, ap_io, 0.2, None, A.mult)
                nc.vector.tensor_tensor(ap_io, ap_io, scratch, A.max)

            for (b0, b1, qr) in (chunk_list if STAGE >= 1 else []):
                t0, t1 = qr[0][0], qr[3][1]
                TB = t1 - t0
                g1 = g1p.tile([128, TB, 256], BF16, tag="g1")
                for qv, (qs, qe) in enumerate(qr):
                    if qe == qs:
                        continue
                    nt = qe - qs
                    ix = adp.tile([128, 8 * TB], mybir.dt.int16, tag="ix")
                    nc.sync.dma_start(ix[:, : 8 * nt], sr16_d[:, 8 * qs : 8 * qe])
                    if GATH & 1:
                        gather_split(g1, t1glob[qv * QRC : (qv + 1) * QRC, :],
                                     ix, qs, qe, t0, qs)
                ixd = adp.tile([128, 8 * TB], mybir.dt.int16, tag="ixd")
                nc.sync.dma_start(ixd[:], dl16_d[:, 8 * t0 : 8 * t1])
                ad = adp.tile([128, TB, 128], BF16, tag="ad")
                if GATH & 2:
                    gather_split(ad, adst1g[:, :], ixd, t0, t1, t0, t0)
                al = alp.tile([128, TB, 4], F32, tag="al")
                scr = alp.tile([128, TB, 4], F32, tag="scr")
                exb = alp.tile([128, TB, 4], BF16, tag="exb")
                if STAGE < 2:
                    continue
                nc.vector.tensor_copy(al[:], g1[:, :, 128:132])  # a_srcE (bf16->f32)
                nc.vector.tensor_copy(scr[:], ad[:, :, 0:4])
                nc.vector.tensor_tensor(al[:], al[:], scr[:], A.add)
                ea_sl = ea_t[:, t0:t1]
                ea_b = _ap(ea_sl, [ea_sl.ap[0], [ea_sl.ap[1][0], TB], [0, 4]])
                we1_b = _ap(we1_t[:, :], [we1_t[:, :].ap[0], [0, TB], [1, 4]])
                nc.vector.tensor_tensor(scr[:], ea_b, we1_b, A.mult)
                nc.vector.tensor_tensor(al[:], al[:], scr[:], A.add)
                lrelu(al[:], scr[:])
                nc.scalar.activation(al[:], al[:], ACTF.Exp)
                nc.vector.tensor_copy(exb[:], al[:])
                rhs = rhsp.tile([128, TB, 134], BF16, tag="rhs")
                if STAGE < 3:
                    continue
                exb_ap = exb[:, :, :]
                exb_b = _ap(exb_ap, list(exb_ap.ap) + [[0, 32]])
                nc.vector.tensor_tensor(
                    rhs[:, :, 0:128].rearrange("p t (h c) -> p t h c", h=4),
                    g1[:, :, 0:128].rearrange("p t (h c) -> p t h c", h=4),
                    exb_b, A.mult)
                nc.vector.tensor_copy(rhs[:, :, 128:132], exb[:])
                nc.vector.tensor_copy(rhs[:, :, 132:133],
                                      _ap(ea_sl, [ea_sl.ap[0], [ea_sl.ap[1][0], TB], [0, 1]]))
                nc.vector.memset(rhs[:, :, 133:134], 1.0)

                pall = epi.tile([128, b1 - b0, 134], F32, tag="pall")
                for b in (range(b0, b1) if STAGE >= 4 else []):
                    pB = ps0.tile([128, 134], F32, space="PSUM", tag="pB")
                    tiles = [t for (ts, te) in blk_segs[b] for t in range(ts, te)]
                    assert tiles, f"block {b} has no tiles"
                    for i, t in enumerate(tiles):
                        P = pp.tile([128, 128], BF16, tag="P")
                        nc.vector.tensor_scalar(
                            P[:], iota_t[:], low_t[:, t : t + 1], None, A.is_equal)
                        nc.tensor.matmul(
                            pB[:], P[:], rhs[:, t - t0, :],
                            start=(i == 0), stop=(i == len(tiles) - 1))

                    # ---- stash psum for batched epilogue ----
                    nc.vector.tensor_copy(
                        pall[:, b - b0, :], pB[:])

                # ---- batched L1 epilogue for the chunk ----
                if STAGE < 4:
                    continue
                NB = b1 - b0
                den = pall[:, :, 128:132]
                degc = epi.tile([128, NB, 1], F32, tag="degc")
                nc.vector.tensor_scalar(degc[:], pall[:, :, 133:134], 1.0, None, A.max)
                nc.vector.reciprocal(degc[:], degc[:])
                la2d = loop_sb[:, b0:b1]
                la3 = _ap(la2d, list(la2d.ap) + [[1, 1]])
                nc.vector.tensor_tensor(la3, pall[:, :, 132:133], degc[:], A.mult)
                alc = epi.tile([128, NB, 4], F32, tag="alc")
                scr4 = epi.tile([128, NB, 4], F32, tag="scr4")
                nc.vector.tensor_tensor(alc[:], asrc1_sb[:, b0:b1, :], adst1_sb[:, b0:b1, :], A.add)
                la_b = _ap(la3, [la3.ap[0], la3.ap[1], [0, 4]])
                we1_b3 = _ap(we1_t[:, :], [we1_t[:, :].ap[0], [0, NB], [1, 4]])
                nc.vector.tensor_tensor(scr4[:], la_b, we1_b3, A.mult)
                nc.vector.tensor_tensor(alc[:], alc[:], scr4[:], A.add)
                lrelu(alc[:], scr4[:])
                nc.scalar.activation(alc[:], alc[:], ACTF.Exp)
                dent = epi.tile([128, NB, 4], F32, tag="dent")
                nc.vector.tensor_tensor(dent[:], den, alc[:], A.add)
                rec = epi.tile([128, NB, 4], F32, tag="rec")
                nc.vector.reciprocal(rec[:], dent[:])
                exlb = epi.tile([128, NB, 4], BF16, tag="exlb")
                nc.vector.tensor_copy(exlb[:], alc[:])
                hb = epi.tile([128, NB, 128], BF16, tag="hb")
                nc.sync.dma_start(
                    hb[:], t1loc[128 * b0 : 128 * b1, 0:128].rearrange(
                        "(a p) c -> p a c", p=128))
                num = epi.tile([128, NB, 128], F32, tag="num")
                exlb_b = _ap(exlb[:, :, :], list(exlb[:, :, :].ap) + [[0, 32]])
                nc.vector.tensor_tensor(
                    num[:].rearrange("p b (h c) -> p b h c", h=4),
                    hb[:].rearrange("p b (h c) -> p b h c", h=4), exlb_b, A.mult)
                nc.vector.tensor_tensor(num[:], num[:], pall[:, :, 0:128], A.add)
                rec_b = _ap(rec[:, :, :], list(rec[:, :, :].ap) + [[0, 32]])
                o1 = epi.tile([128, NB, 128], F32, tag="o1")
                nc.vector.tensor_tensor(
                    o1[:].rearrange("p b (h c) -> p b h c", h=4),
                    num[:].rearrange("p b (h c) -> p b h c", h=4), rec_b, A.mult)
                b1_b3 = _ap(b1_bc[:, :], [b1_bc[:, :].ap[0], [0, NB], [1, 128]])
                nc.vector.tensor_tensor(o1[:], o1[:], b1_b3, A.add)
                mn = epi.tile([128, NB, 128], F32, tag="mn")
                nc.vector.tensor_scalar(mn[:], o1[:], 0.0, None, A.min)
                nc.scalar.activation(mn[:], mn[:], ACTF.Exp)
                g = epi.tile([128, NB, 128], F32, tag="g")
                nc.vector.tensor_scalar(g[:], o1[:], 0.0, None, A.max)
                nc.vector.tensor_tensor(g[:], g[:], mn[:], A.add)
                nc.vector.tensor_scalar(g[:], g[:], -1.0, None, A.add)
                gscr = epi.tile([128, NB, 128], F32, tag="gscr")
                v_b3 = _ap(v_bc[:, :], [v_bc[:, :].ap[0], [0, NB], [1, 128]])
                ws2_b3 = _ap(ws2_bc[:, :], [ws2_bc[:, :].ap[0], [0, NB], [1, 128]])
                wd2_b3 = _ap(wd2_bc[:, :], [wd2_bc[:, :].ap[0], [0, NB], [1, 128]])
                nc.vector.tensor_tensor(gscr[:], g[:], v_b3, A.mult)
                nc.vector.tensor_reduce(
                    t2_sb[:, b0:b1, 0:1], gscr[:], op=A.add, axis=mybir.AxisListType.X)
                nc.vector.tensor_tensor(gscr[:], g[:], ws2_b3, A.mult)
                nc.vector.tensor_reduce(
                    t2_sb[:, b0:b1, 1:2], gscr[:], op=A.add, axis=mybir.AxisListType.X)
                nc.vector.tensor_tensor(gscr[:], g[:], wd2_b3, A.mult)
                ad2d = adst2_sb[:, b0:b1]
                nc.vector.tensor_reduce(
                    _ap(ad2d, list(ad2d.ap) + [[1, 1]]), gscr[:],
                    op=A.add, axis=mybir.AxisListType.X)
                t2row = epi.tile([128, NB, 128], BF16, tag="t2row")
                nc.vector.memset(t2row[:], 0.0)
                nc.vector.tensor_copy(t2row[:, :, 0:2], t2_sb[:, b0:b1, :])
                nc.sync.dma_start(
                    t2loc[128 * b0 : 128 * b1, :].rearrange("(a p) c -> p a c", p=128),
                    t2row[:])
                a2row = epi.tile([128, NB, 128], BF16, tag="a2row")
                nc.vector.memset(a2row[:], 0.0)
                ad2d2 = adst2_sb[:, b0:b1]
                nc.vector.tensor_copy(
                    a2row[:, :, 0:1], _ap(ad2d2, list(ad2d2.ap) + [[1, 1]]))
                nc.sync.dma_start(
                    adst2g[128 * b0 : 128 * b1, :].rearrange("(a p) c -> p a c", p=128),
                    a2row[:])

            # ======== layer 2 ========
            if SIM_NOCOLL:
                nc.gpsimd.dma_start(t2glob[0:NPAD, :], t2loc[:, :])
            else:
                nc.gpsimd.collective_compute(
                    "AllGather", mybir.AluOpType.bypass,
                    replica_groups=[list(range(NC))],
                    ins=[t2loc.ap().opt()], outs=[t2glob.ap().opt()])

            for (b0, b1, qr) in (chunk_list if STAGE >= 5 else []):
                t0, t1 = qr[0][0], qr[3][1]
                TB = t1 - t0
                pr = g1p.tile([128, TB, 128], BF16, tag="g1")
                for qv, (qs, qe) in enumerate(qr):
                    if qe == qs:
                        continue
                    nt = qe - qs
                    ix2 = adp.tile([128, 8 * TB], mybir.dt.int16, tag="ix")
                    nc.sync.dma_start(ix2[:, : 8 * nt], sr16_d[:, 8 * qs : 8 * qe])
                    gather_split(pr, t2glob[qv * QRC : (qv + 1) * QRC, :],
                                 ix2, qs, qe, t0, qs)
                ixd2 = adp.tile([128, 8 * TB], mybir.dt.int16, tag="ixd")
                nc.sync.dma_start(ixd2[:], dl16_d[:, 8 * t0 : 8 * t1])
                ad2 = adp.tile([128, TB, 128], BF16, tag="ad")
                gather_split(ad2, adst2g[:, :], ixd2, t0, t1, t0, t0)
                al2 = alp.tile([128, TB], F32, tag="al")
                scr2 = alp.tile([128, TB], F32, tag="scr")
                if STAGE < 6:
                    continue
                nc.vector.tensor_copy(al2[:], pr[:, :, 1])
                nc.vector.tensor_copy(scr2[:], ad2[:, :, 0])
                nc.vector.tensor_tensor(al2[:], al2[:], scr2[:], A.add)
                nc.vector.tensor_scalar(scr2[:], ea_t[:, t0:t1], we2_bc[:, :1], None, A.mult)
                nc.vector.tensor_tensor(al2[:], al2[:], scr2[:], A.add)
                lrelu(al2[:], scr2[:])
                nc.scalar.activation(al2[:], al2[:], ACTF.Exp)
                rhs2 = rhsp.tile([128, TB, 2], BF16, tag="rhs")
                nc.vector.tensor_copy(rhs2[:, :, 0:1], pr[:, :, 0:1])
                nc.vector.memset(rhs2[:, :, 1:2], 1.0)

                p2all = epi.tile([128, b1 - b0, 2], F32, tag="p2all")
                if STAGE < 7:
                    continue
                for b in range(b0, b1):
                    pB2 = ps0.tile([128, 2], F32, space="PSUM", tag="pB2")
                    tiles = [t for (ts, te) in blk_segs[b] for t in range(ts, te)]
                    for i, t in enumerate(tiles):
                        P2 = pp.tile([128, 128], BF16, tag="P")
                        nc.vector.tensor_scalar(
                            P2[:], iota_t[:], low_t[:, t : t + 1],
                            al2[:, t - t0 : t - t0 + 1], A.is_equal, A.mult)
                        nc.tensor.matmul(
                            pB2[:], P2[:], rhs2[:, t - t0, :],
                            start=(i == 0), stop=(i == len(tiles) - 1))

                    nc.vector.tensor_copy(p2all[:, b - b0, :], pB2[:])

                # ---- batched L2 epilogue ----
                NB = b1 - b0
                al1 = epi.tile([128, NB, 1], F32, tag="al1")
                s1 = epi.tile([128, NB, 1], F32, tag="s1")
                nc.vector.tensor_tensor(
                    al1[:], t2_sb[:, b0:b1, 1:2],
                    _ap(adst2_sb[:, b0:b1], list(adst2_sb[:, b0:b1].ap) + [[1, 1]]), A.add)
                we2_b3 = _ap(we2_bc[:, :], [we2_bc[:, :].ap[0], [0, NB], [1, 1]])
                nc.vector.tensor_tensor(
                    s1[:], _ap(loop_sb[:, b0:b1], list(loop_sb[:, b0:b1].ap) + [[1, 1]]),
                    we2_b3, A.mult)
                nc.vector.tensor_tensor(al1[:], al1[:], s1[:], A.add)
                lrelu(al1[:], s1[:])
                nc.scalar.activation(al1[:], al1[:], ACTF.Exp)
                numt = epi.tile([128, NB, 1], F32, tag="numt")
                nc.vector.tensor_tensor(numt[:], al1[:], t2_sb[:, b0:b1, 0:1], A.mult)
                nc.vector.tensor_tensor(numt[:], numt[:], p2all[:, :, 0:1], A.add)
                dent2 = epi.tile([128, NB, 1], F32, tag="dent2")
                nc.vector.tensor_tensor(dent2[:], p2all[:, :, 1:2], al1[:], A.add)
                nc.vector.reciprocal(dent2[:], dent2[:])
                fin = _ap(out_sb[:, b0:b1], list(out_sb[:, b0:b1].ap) + [[1, 1]])
                nc.vector.tensor_tensor(fin, numt[:], dent2[:], A.mult)
                c_b3 = _ap(c_bc[:, :], [c_bc[:, :].ap[0], [0, NB], [1, 1]])
                nc.vector.tensor_tensor(fin, fin, c_b3, A.add)

            with nc.allow_non_contiguous_dma(reason="block-major output"):
                nc.sync.dma_start(
                    out_d[:, 0].rearrange("(b p) -> p b", p=128), out_sb[:])

    nc.compile()
    return nc


def _make_in_maps(inputs, shards):
    x = np.asarray(inputs["x"], np.float32)
    xTp = np.zeros((NC, 128, NPAD), np.float32)
    for c in range(NC):
        xTp[c, :, :NLOC] = x[c * NLOC : (c + 1) * NLOC].T
    iota = np.tile(np.arange(128, dtype=np.float32), (128, 1)).astype(BF)
    ident = np.eye(128, dtype=np.float32)
    common = dict(
        iota=iota, ident=ident,
        W1=np.asarray(inputs["W1"], np.float32),
        as1f=np.asarray(inputs["as1"], np.float32).reshape(1, 128),
        ad1f=np.asarray(inputs["ad1"], np.float32).reshape(1, 128),
        We1f=np.asarray(inputs["We1"], np.float32).reshape(1, 128),
        ae1f=np.asarray(inputs["ae1"], np.float32).reshape(1, 128),
        b1f=np.asarray(inputs["b1"], np.float32).reshape(1, 128),
        W2=np.asarray(inputs["W2"], np.float32),
        as2f=np.asarray(inputs["as2"], np.float32).reshape(1, 64),
        ad2f=np.asarray(inputs["ad2"], np.float32).reshape(1, 64),
        We2f=np.asarray(inputs["We2"], np.float32).reshape(1, 64),
        ae2f=np.asarray(inputs["ae2"], np.float32).reshape(1, 64),
        b2f=np.asarray(inputs["b2"], np.float32).reshape(1, 64),
        fcWT=np.asarray(inputs["fcW"], np.float32).reshape(1, 64),
        fcb=np.asarray(inputs["fcb"], np.float32).reshape(1, 1),
    )
    return [dict(common, xT=xTp[c], sr16=shards[c]["sr16"], dl16=shards[c]["dl16"],
                 low=shards[c]["low"], ea=shards[c]["ea"]) for c in range(NC)]


def kernel(x, edge_index, edge_attr, W1, as1, ad1, We1, ae1, b1,
           W2, as2, ad2, We2, ae2, b2, fcW, fcb):
    shards, meta = _build_shards(np.asarray(edge_index), np.asarray(edge_attr))
    nc = _build_graph(meta)
    in_maps = _make_in_maps(
        dict(x=x, W1=W1, as1=as1, ad1=ad1, We1=We1, ae1=ae1, b1=b1, W2=W2,
             as2=as2, ad2=ad2, We2=We2, ae2=ae2, b2=b2, fcW=fcW, fcb=fcb),
        shards)
    res = run_bass_kernel_spmd(nc, in_maps, core_ids=list(range(NC)))
    out = np.concatenate(
        [res.results[c]["out"][:NLOC, :] for c in range(NC)], axis=0)
    return out.astype(np.float32)

